# revision 2
# baseline (speedup 1.0000x reference)
"""v2 Trainium2 Bass kernel for nn_Head.

Math (per batch b):
    Q = X @ Wq + bq                      [T, D]
    S = Q Q^T / 8                        [T, T]   (symmetric)
    W = softmax(S, axis=0)  (normalize over rows i per column j)
    A[i, d] = sum_j W[i, j] Q[j, d]

Because S is symmetric, column-softmax stats are row stats:
    E[j, i] = exp(S[j, i] / 8), l_j = sum_i E[j, i]
    A^T = sum_J Qs_J^T @ E_J with Qs = Q / l

Key implementation choices (driven by measured backend cost structure):
  - ALL matmul operands bf16 (f32r matmuls measured ~7x slower).
  - x loaded with a single SWDGE cast DMA per batch into natural layout
    with a PERMUTED t-axis: SBUF partition p holds rows 16p..16p+15, so the
    DMA is one contiguous 32KB run per partition. All compute runs in the
    permuted order (it's a global permutation of t, harmless), and the final
    store un-permutes for free: partition p stores rows 16p..16p+15 as one
    contiguous 4KB run.
  - X^T via PE transposes (4 per PSUM bank, one DVE evacuation per 4).
  - Single output store per batch.

Sharding: data-parallel over batch, 2 batches/core, 8 cores, no collectives.
"""

import numpy as np

import concourse.bass as bass
import concourse.mybir as mybir
import concourse.tile as tile
from concourse.bass import ds, ts
from concourse.bass_utils import run_bass_kernel_spmd
from concourse.masks import make_identity

B, T, E, D = 16, 2048, 512, 64
NCORES = 8
BPC = B // NCORES
P = 128
NJ = T // P  # 16
NCH = T // 512  # 4
KO = E // P  # 4

f32 = mybir.dt.float32
bf16 = mybir.dt.bfloat16
EXP = mybir.ActivationFunctionType.Exp


def split_multi_waits(nc: bass.Bass) -> int:
    import bass_rust

    n_split = 0
    for f in nc.m.functions:
        for blk in f.blocks:
            insts = blk.instructions
            if not any(
                i.sync_info is not None and len(i.sync_info.on_wait) > 1
                for i in insts
            ):
                continue
            new_list = []
            for ins in insts:
                si = ins.sync_info
                if si is not None and len(si.on_wait) > 1:
                    waits = list(si.on_wait)
                    for k, w in enumerate(waits[:-1]):
                        e = mybir.InstEventSemaphore(
                            name=f"wsplit_{ins.name}_{k}", ins=[], outs=[]
                        )
                        e.engine = ins.engine
                        e.sync_info = bass_rust.SyncInfo(on_wait=[w], on_update=[])
                        new_list.append(e)
                        n_split += 1
                    si.on_wait = waits[-1:]
                new_list.append(ins)
            blk.instructions = new_list
    return n_split


def build_module(reps: int = 1) -> bass.Bass:
    nc = bass.Bass("TRN2", target_bir_lowering=False, debug=False, num_devices=NCORES)
    x = nc.declare_dram_parameter("x", [BPC, T, E], f32, isOutput=False).ap()
    wq = nc.declare_dram_parameter("Wq", [E, D], f32, isOutput=False).ap()
    bq = nc.declare_dram_parameter("bq", [D], f32, isOutput=False).ap()
    out = nc.declare_dram_parameter("out", [BPC, T, D], f32, isOutput=True).ap()

    with tile.TileContext(nc) as tc:
        with (
            tc.tile_pool(name="consts", bufs=1) as consts,
            tc.tile_pool(name="xn_p", bufs=2) as xn_p,
            tc.tile_pool(name="xt_p", bufs=2) as xt_p,
            tc.tile_pool(name="qtb_p", bufs=2) as qtb_p,
            tc.tile_pool(name="qn_p", bufs=2) as qn_p,
            tc.tile_pool(name="qs_p", bufs=2) as qs_p,
            tc.tile_pool(name="e_p", bufs=NJ + 1) as e_p,
            tc.tile_pool(name="l_p", bufs=4) as l_p,
            tc.tile_pool(name="at_p", bufs=2) as at_p,
            tc.tile_pool(name="o_p", bufs=2) as o_p,
            tc.tile_pool(name="ps_s", bufs=2, space="PSUM") as ps_s,
            tc.tile_pool(name="ps_m", bufs=2, space="PSUM") as ps_m,
        ):
            # ---- constants ----
            wq_f = consts.tile([P, KO, D], f32)
            nc.gpsimd.dma_start(out=wq_f[:], in_=wq.rearrange("(ko p) d -> p ko d", p=P))
            wq_b = consts.tile([P, KO, D], bf16)
            nc.vector.tensor_copy(wq_b[:], wq_f[:])
            bq_sb = consts.tile([D, 1], f32)
            nc.gpsimd.dma_start(out=bq_sb[:], in_=bq.unsqueeze(1))
            identf = consts.tile([P, P], f32)
            make_identity(nc, identf[:])
            ident = consts.tile([P, P], bf16)
            nc.vector.tensor_copy(ident[:], identf[:])

            for rep in range(reps):
              for b in range(BPC):
                # ---- load X natural (permuted t): xn[p, a, e] = x[16p+a, e] ----
                xn = xn_p.tile([P, NJ, E], bf16, tag="xn", name=f"xn{b}")
                nc.gpsimd.dma_start(
                    out=xn[:], in_=x[b].rearrange("(p a) e -> p a e", p=P)
                )

                # ---- X^T tiles via PE transposes ----
                # xt[p=e%128, ko, k] = X[t=perm(k), e], where column k = a*128+q
                # holds row t = 16q+a  (tile a comes from transposing xn[:, a, :]).
                xt = xt_p.tile([P, KO, T], bf16, tag="xt", name=f"xt{b}")
                for ko in range(KO):
                    for g in range(NJ // 4):  # groups of 4 tiles -> 1 psum bank
                        pt = ps_s.tile([P, 512], bf16, tag="smallb", name=f"px{b}_{ko}_{g}")
                        for u in range(4):
                            a = g * 4 + u
                            nc.tensor.transpose(
                                pt[:, ts(u, P)], xn[:, a, ts(ko, P)], ident[:]
                            )
                        nc.vector.tensor_copy(
                            xt[:, ko, ds(g * 512, 512)], pt[:]
                        )

                # ---- projection: QT[d, k] = sum_e Wq[e, d] X[perm(k), e] + bq ----
                qtb = qtb_p.tile([D, T], bf16, tag="qtb", name=f"qtb{b}")
                for c in range(NCH):
                    ps = ps_s.tile([D, 512], f32, tag="small", name=f"pj{b}_{c}")
                    for ko in range(KO):
                        nc.tensor.matmul(
                            ps[:],
                            lhsT=wq_b[:, ko, :],
                            rhs=xt[:, ko, ts(c, 512)],
                            start=(ko == 0),
                            stop=(ko == KO - 1),
                        )
                    nc.vector.tensor_scalar_add(qtb[:, ts(c, 512)], ps[:], bq_sb[:])

                # ---- Q natural via PE transposes of QT tiles (4 at a time) ----
                qn = qn_p.tile([P, NJ, D], bf16, tag="qn", name=f"qn{b}")
                for g in range(NJ // 4):
                    pt = ps_s.tile([P, 4 * D], bf16, tag="smallb", name=f"ptq{b}_{g}")
                    for u in range(4):
                        j = g * 4 + u
                        nc.tensor.transpose(
                            pt[:, ts(u, D)], qtb[:, ts(j, P)], ident[:D, :D]
                        )
                    nc.vector.tensor_copy(
                        qn[:, ds(g * 4, 4), :].rearrange("p a d -> p (a d)"), pt[:]
                    )

                # ---- phase A: per row-tile J: S = QT_J^T QT, E=exp(S/8), l ----
                la = l_p.tile([P, NJ], f32, tag="l", name=f"la{b}")
                lb = l_p.tile([P, NJ], f32, tag="l", name=f"lb{b}")
                e_tiles = []
                for j in range(NJ):
                    et = e_p.tile([P, T], bf16, tag="E", name=f"e{b}_{j}")
                    e_tiles.append(et)
                    for h, ltile in ((0, la), (1, lb)):
                        ps = ps_m.tile([P, 1024], f32, tag="s", name=f"s{b}_{j}_{h}")
                        for c in range(2):
                            nc.tensor.matmul(
                                ps[:, ts(c, 512)],
                                lhsT=qtb[:, ts(j, P)],
                                rhs=qtb[:, ds(h * 1024 + c * 512, 512)],
                                start=True,
                                stop=True,
                            )
                        nc.scalar.activation(
                            et[:, ds(h * 1024, 1024)],
                            ps[:],
                            EXP,
                            bias=0.0,
                            scale=0.125,
                            accum_out=ltile[:, ds(j, 1)],
                        )
                # r = 1 / (la + lb);  qs = qn * r
                lr = l_p.tile([P, NJ], f32, tag="l", name=f"lr{b}")
                nc.vector.tensor_add(lr[:], la[:], lb[:])
                nc.vector.reciprocal(lr[:], lr[:])
                qs = qs_p.tile([P, NJ, D], bf16, tag="qs", name=f"qs{b}")
                for j in range(NJ):
                    nc.vector.tensor_scalar_mul(
                        qs[:, j, :], qn[:, j, :], lr[:, ds(j, 1)]
                    )

                # ---- phase B: A^T[d, k] = sum_J Qs_J^T @ E_J ----
                at = at_p.tile([D, T], f32, tag="at", name=f"at{b}")
                for c in range(NCH):
                    ps = ps_s.tile([D, 512], f32, tag="small", name=f"pa{b}_{c}")
                    for j in range(NJ):
                        nc.tensor.matmul(
                            ps[:],
                            lhsT=qs[:, j, :],
                            rhs=e_tiles[j][:, ts(c, 512)],
                            start=(j == 0),
                            stop=(j == NJ - 1),
                        )
                    nc.vector.tensor_copy(at[:, ts(c, 512)], ps[:])

                # ---- un-permute: transpose A^T tile i -> rows {16p+i} ----
                # ot[p, i, d] = A[16p+i, d]; tile i of at (columns 128i..128i+127)
                # holds rows t=16q+i at column q.
                ot = o_p.tile([P, NJ, D], f32, tag="o", name=f"o{b}")
                for g in range(NJ // 4):
                    pt = ps_s.tile([P, 4 * D], f32, tag="small", name=f"pto{b}_{g}")
                    for u in range(4):
                        i = g * 4 + u
                        nc.tensor.transpose(
                            pt[:, ts(u, D)], at[:, ts(i, P)], identf[:D, :D]
                        )
                    nc.vector.tensor_copy(
                        ot[:, ds(g * 4, 4), :].rearrange("p a d -> p (a d)"), pt[:]
                    )
                nc.sync.dma_start(
                    out=out[b].rearrange("(p a) d -> p a d", p=P), in_=ot[:]
                )

    split_multi_waits(nc)
    return nc


def kernel(x: np.ndarray, Wq: np.ndarray, bq: np.ndarray) -> np.ndarray:
    assert x.shape == (B, T, E) and Wq.shape == (E, D) and bq.shape == (D,)
    nc = build_module()
    in_maps = [
        {
            "x": np.ascontiguousarray(x[i * BPC : (i + 1) * BPC]),
            "Wq": np.ascontiguousarray(Wq),
            "bq": np.ascontiguousarray(bq),
        }
        for i in range(NCORES)
    ]
    res = run_bass_kernel_spmd(nc, in_maps, core_ids=list(range(NCORES)))
    return np.concatenate([res.results[i]["out"] for i in range(NCORES)], axis=0)


# revision 3
# speedup vs baseline: 1.3316x; 1.3316x over previous
"""Trainium2 Bass kernel for nn_Head (final).

Like v2/v3 (see kernel_v2.py docstring for the math) but obtains X^T via a
single DRAM->DRAM f32->bf16 cast + ONE xbar DMA-transposed load covering both
batches (natural t order), removing the 256 PE transposes + 64 evacuation
copies of the cast-load path. The output un-transpose uses strided column
slices of A^T (at[:, i::16]) so partition p receives rows 16p+i, making the
final store one contiguous 4KB run per partition, one DMA for both batches.
"""

import numpy as np

import concourse.bass as bass
import concourse.mybir as mybir
import concourse.tile as tile
from concourse.bass import ds, ts
from concourse.bass_utils import run_bass_kernel_spmd
from concourse.masks import make_identity

B, T, E, D = 16, 2048, 512, 64
NCORES = 8
BPC = B // NCORES
P = 128
NJ = T // P  # 16
NCH = T // 512  # 4
KO = E // P  # 4

f32 = mybir.dt.float32
bf16 = mybir.dt.bfloat16
EXP = mybir.ActivationFunctionType.Exp


def split_multi_waits(nc: bass.Bass) -> int:
    import bass_rust

    n_split = 0
    for f in nc.m.functions:
        for blk in f.blocks:
            insts = blk.instructions
            if not any(
                i.sync_info is not None and len(i.sync_info.on_wait) > 1
                for i in insts
            ):
                continue
            new_list = []
            for ins in insts:
                si = ins.sync_info
                if si is not None and len(si.on_wait) > 1:
                    waits = list(si.on_wait)
                    for k, w in enumerate(waits[:-1]):
                        e = mybir.InstEventSemaphore(
                            name=f"wsplit_{ins.name}_{k}", ins=[], outs=[]
                        )
                        e.engine = ins.engine
                        e.sync_info = bass_rust.SyncInfo(on_wait=[w], on_update=[])
                        new_list.append(e)
                        n_split += 1
                    si.on_wait = waits[-1:]
                new_list.append(ins)
            blk.instructions = new_list
    return n_split


def build_module(reps: int = 1) -> bass.Bass:
    nc = bass.Bass("TRN2", target_bir_lowering=False, debug=False, num_devices=NCORES)
    x = nc.declare_dram_parameter("x", [BPC, T, E], f32, isOutput=False).ap()
    wq = nc.declare_dram_parameter("Wq", [E, D], f32, isOutput=False).ap()
    bq = nc.declare_dram_parameter("bq", [D], f32, isOutput=False).ap()
    out = nc.declare_dram_parameter("out", [BPC, T, D], f32, isOutput=True).ap()
    xbf = nc.dram_tensor("xbf", [BPC, T, E], bf16).ap()

    with tile.TileContext(nc) as tc:
        with (
            tc.tile_pool(name="consts", bufs=1) as consts,
            tc.tile_pool(name="xt_p", bufs=1) as xt_p,
            tc.tile_pool(name="qtb_p", bufs=2) as qtb_p,
            tc.tile_pool(name="qn_p", bufs=2) as qn_p,
            tc.tile_pool(name="qs_p", bufs=2) as qs_p,
            tc.tile_pool(name="e_p", bufs=NJ) as e_p,
            tc.tile_pool(name="l_p", bufs=4) as l_p,
            tc.tile_pool(name="at_p", bufs=2) as at_p,
            tc.tile_pool(name="o_p", bufs=1) as o_p,
            tc.tile_pool(name="ps_s", bufs=2, space="PSUM") as ps_s,
            tc.tile_pool(name="ps_m", bufs=1, space="PSUM") as ps_m,
        ):
            # ---- constants ----
            wq_f = consts.tile([P, KO, D], f32)
            nc.gpsimd.dma_start(out=wq_f[:], in_=wq.rearrange("(ko p) d -> p ko d", p=P))
            wq_b = consts.tile([P, KO, D], bf16)
            nc.vector.tensor_copy(wq_b[:], wq_f[:])
            bq_sb = consts.tile([D, 1], f32)
            nc.gpsimd.dma_start(out=bq_sb[:], in_=bq.unsqueeze(1))
            identf = consts.tile([D, D], f32)
            make_identity(nc, identf[:])
            ident = consts.tile([D, D], bf16)
            nc.vector.tensor_copy(ident[:], identf[:])

            for rep in range(reps):
              # one cast + one transposed load for BOTH batches
              nc.gpsimd.dma_start(out=xbf[:], in_=x[:])
              xt2 = xt_p.tile([P, KO, BPC * T], bf16, tag="xt", name=f"xt{rep}")
              nc.sync.dma_start(
                  out=xt2[:], in_=xbf.rearrange("b t e -> (b t) e"), transpose=True
              )
              ot2 = o_p.tile([P, BPC, NJ, D], f32, tag="o", name=f"ot{rep}")
              for b in range(BPC):
                xt = xt2[:, :, ds(b * T, T)]

                # ---- projection: QT[d, t] = sum_e Wq[e, d] X[t, e] + bq ----
                qtb = qtb_p.tile([D, T], bf16, tag="qtb", name=f"qtb{b}")
                for c in range(NCH):
                    ps = ps_s.tile([D, 512], f32, tag="small", name=f"pj{b}_{c}")
                    for ko in range(KO):
                        nc.tensor.matmul(
                            ps[:],
                            lhsT=wq_b[:, ko, :],
                            rhs=xt[:, ko, ts(c, 512)],
                            start=(ko == 0),
                            stop=(ko == KO - 1),
                        )
                    nc.vector.tensor_scalar_add(qtb[:, ts(c, 512)], ps[:], bq_sb[:])

                # ---- Q natural via PE transposes of QT tiles (4 at a time) ----
                qn = qn_p.tile([P, NJ, D], bf16, tag="qn", name=f"qn{b}")
                for g in range(NJ // 4):
                    pt = ps_s.tile([P, 4 * D], bf16, tag="smallb", name=f"ptq{b}_{g}")
                    for u in range(4):
                        j = g * 4 + u
                        nc.tensor.transpose(
                            pt[:, ts(u, D)], qtb[:, ts(j, P)], ident[:]
                        )
                    nc.vector.tensor_copy(
                        qn[:, ds(g * 4, 4), :].rearrange("p a d -> p (a d)"), pt[:]
                    )

                # ---- phase A: per row-tile J: S = QT_J^T QT, E=exp(S/8), l ----
                la = l_p.tile([P, NJ], f32, tag="l", name=f"la{b}")
                e_tiles = []
                for j in range(NJ):
                    et = e_p.tile([P, T], bf16, tag="E", name=f"e{b}_{j}")
                    e_tiles.append(et)
                    ps = ps_m.tile([P, T], f32, tag="s", name=f"s{b}_{j}")
                    for c in range(NCH):
                        nc.tensor.matmul(
                            ps[:, ts(c, 512)],
                            lhsT=qtb[:, ts(j, P)],
                            rhs=qtb[:, ts(c, 512)],
                            start=True,
                            stop=True,
                        )
                    nc.scalar.activation(
                        et[:],
                        ps[:],
                        EXP,
                        bias=0.0,
                        scale=0.125,
                        accum_out=la[:, ds(j, 1)],
                    )
                # r = 1/l;  qs = qn * r (broadcast mul over d)
                lr = l_p.tile([P, NJ], f32, tag="l", name=f"lr{b}")
                nc.vector.reciprocal(lr[:], la[:])
                qs = qs_p.tile([P, NJ, D], bf16, tag="qs", name=f"qs{b}")
                nc.vector.tensor_mul(
                    qs[:],
                    qn[:],
                    lr[:].unsqueeze(2).broadcast_to([P, NJ, D]),
                )

                # ---- phase B: A^T[d, t] = sum_J Qs_J^T @ E_J ----
                at = at_p.tile([D, T], f32, tag="at", name=f"at{b}")
                for c in range(NCH):
                    ps = ps_s.tile([D, 512], f32, tag="small", name=f"pa{b}_{c}")
                    for j in range(NJ):
                        nc.tensor.matmul(
                            ps[:],
                            lhsT=qs[:, j, :],
                            rhs=e_tiles[j][:, ts(c, 512)],
                            start=(j == 0),
                            stop=(j == NJ - 1),
                        )
                    nc.vector.tensor_copy(at[:, ts(c, 512)], ps[:])

                # ---- un-transpose with strided slices: ot2[p, b, i, d] = A[16p+i, d]
                at_aq = at[:].rearrange("d (q a) -> d a q", a=NJ)
                for g in range(NJ // 4):
                    pt = ps_s.tile([P, 4 * D], f32, tag="small", name=f"pto{b}_{g}")
                    for u in range(4):
                        i = g * 4 + u
                        nc.tensor.transpose(
                            pt[:, ts(u, D)], at_aq[:, i, :], identf[:]
                        )
                    nc.vector.tensor_copy(
                        ot2[:, b, ds(g * 4, 4), :].rearrange("p a d -> p (a d)"),
                        pt[:],
                    )
              nc.sync.dma_start(
                  out=out.rearrange("b (p a) d -> p b a d", p=P), in_=ot2[:]
              )

    split_multi_waits(nc)
    return nc


def kernel(x: np.ndarray, Wq: np.ndarray, bq: np.ndarray) -> np.ndarray:
    assert x.shape == (B, T, E) and Wq.shape == (E, D) and bq.shape == (D,)
    nc = build_module()
    in_maps = [
        {
            "x": np.ascontiguousarray(x[i * BPC : (i + 1) * BPC]),
            "Wq": np.ascontiguousarray(Wq),
            "bq": np.ascontiguousarray(bq),
        }
        for i in range(NCORES)
    ]
    res = run_bass_kernel_spmd(nc, in_maps, core_ids=list(range(NCORES)))
    return np.concatenate([res.results[i]["out"] for i in range(NCORES)], axis=0)
